# revision 1
# baseline (speedup 1.0000x reference)
"""Trainium2 Bass kernel for nn_DARTSModel — self-contained submission.

kernel(**inputs) takes FULL unsharded inputs (numpy), shards batch across
8 NeuronCores (data parallel), runs the Bass kernel via PJRT, gathers.
"""
import sys
sys.path.insert(0, "/opt/trn_rl_repo")

import numpy as np
from contextlib import ExitStack

import concourse.bass as bass
import concourse.tile as tile
from concourse import bacc, mybir

F32R = mybir.dt.float32r
BF16 = mybir.dt.bfloat16
F32 = mybir.dt.float32
DT = F32R   # main compute dtype (states, x, W0)
WSDT = BF16  # Ws dtype (SBUF capacity)
AF = mybir.ActivationFunctionType

EMB, HID, IN_DIM = 300, 512, 360
NJS = 2 * HID  # 1024
CONNECTIONS = [("tanh", 0), ("relu", 1), ("tanh", 1), ("relu", 0),
               ("identity", 2), ("sigmoid", 3), ("tanh", 4), ("relu", 5)]
ACT_FN = {"tanh": AF.Tanh, "relu": AF.Relu, "sigmoid": AF.Sigmoid}

# DAG levels: lists of connection indices (state s_{i+1} = g(states[conn_i], Ws[i]))
LEVELS = [[0, 3], [1, 2, 6], [4, 5], [7]]
# which states need a k-layout transpose (feed a later matmul): s0..s5
NEEDS_T = [True, True, True, True, True, True, False, False, False]
# state index -> (stack, band): s1,s3,s5,s7 -> stack A bands 0..3; s2,s4,s6,s8 -> stack B
def stack_pos(si):  # si in 1..8
    k = si - 1
    return (k % 2, (k // 2) * 32)  # (stack id, partition offset)

# W0 row chunking: x part rows 0:300 ([128,128,44]), h part rows 300:812 (4x128)
XCH = [(0, 128), (128, 128), (256, 44)]
HCH = [(300 + 128 * i, 128) for i in range(4)]


def build(nc, B=16, T=256, n_chunk=256):
    """Emit the kernel into nc (a Bacc). n_chunk: matmul N tile (256 or 512)."""
    assert 128 % B == 0 and B <= 32
    BT = B * T
    BTP = BT + B                   # padded per-chunk xT width (t-major slices read 32 cols)
    MW = 2 * B                     # stationary operand width (col group = 32)
    NG = NJS // n_chunk            # col groups used per js matmul round
    dt = DT

    # ---- DRAM I/O ----
    inT = nc.dram_tensor("inputs_T", [IN_DIM, BT], dt, kind="ExternalInput").ap()
    masks = nc.dram_tensor("masks", [B, T], F32, kind="ExternalInput").ap()
    wenc_d = nc.dram_tensor("W_enc", [IN_DIM, EMB], dt, kind="ExternalInput").ap()
    benc_d = nc.dram_tensor("b_enc", [EMB], F32, kind="ExternalInput").ap()
    w0_d = nc.dram_tensor("W0", [EMB + HID, NJS], dt, kind="ExternalInput").ap()
    ws_d = nc.dram_tensor("Ws", [8, HID, NJS], WSDT, kind="ExternalInput").ap()
    ident_d = nc.dram_tensor("ident", [B, B], dt, kind="ExternalInput").ap()
    identb_d = nc.dram_tensor("ident_bf", [128, 128], WSDT, kind="ExternalInput").ap()
    # EA/EB: [128, B] selector matrices for the mean (1/8 at [32k+b, b])
    ea_d = nc.dram_tensor("EA", [128, B], dt, kind="ExternalInput").ap()
    zeros_d = nc.dram_tensor("zeros", [128, HID], dt, kind="ExternalInput").ap()
    out_d = nc.dram_tensor("out", [B, T, HID], F32, kind="ExternalOutput").ap()

    ctx = nc._build_ctx  # set by caller
    tc = nc._build_tc

    wp = ctx.enter_context(tc.tile_pool(name="weights", bufs=1))
    sp = ctx.enter_context(tc.tile_pool(name="state", bufs=1))
    xp = ctx.enter_context(tc.tile_pool(name="xenc", bufs=1))
    pp = ctx.enter_context(tc.tile_pool(name="psum", bufs=2, space="PSUM"))
    pjs = ctx.enter_context(tc.tile_pool(name="psum_js", bufs=2, space="PSUM"))
    gp = ctx.enter_context(tc.tile_pool(name="gate", bufs=2))
    op = ctx.enter_context(tc.tile_pool(name="outs", bufs=3))

    # ---- load weights into SBUF ----
    w0_sb = wp.tile([128, 7 * NJS], dt, tag="w0")          # 7 row-chunks side by side
    for c, (r0, rn) in enumerate(XCH + HCH):
        nc.sync.dma_start(w0_sb[0:rn, c * NJS:(c + 1) * NJS], w0_d[r0:r0 + rn, :])
    ws_sb = wp.tile([128, 32 * NJS], WSDT, tag="ws")         # (i,c) at col (i*4+c)*NJS
    for i in range(8):
        for c in range(4):
            nc.sync.dma_start(ws_sb[:, (i * 4 + c) * NJS:(i * 4 + c + 1) * NJS],
                              ws_d[i, 128 * c:128 * (c + 1), :])
    we_sb = wp.tile([128, 3 * EMB], dt, tag="wenc")
    for c, (r0, rn) in enumerate([(0, 128), (128, 128), (256, 104)]):
        nc.sync.dma_start(we_sb[0:rn, c * EMB:(c + 1) * EMB], wenc_d[r0:r0 + rn, :])
    benc_sb = wp.tile([128, 3], F32, tag="benc")            # [300] as 3 col chunks
    for c, (r0, rn) in enumerate([(0, 128), (128, 128), (256, 44)]):
        nc.sync.dma_start(benc_sb[0:rn, c:c + 1], benc_d[r0:r0 + rn].rearrange("(p o) -> p o", o=1))
    ident = wp.tile([B, B], dt, tag="ident")
    nc.sync.dma_start(ident[:], ident_d[:])
    identb = wp.tile([128, 128], WSDT, tag="identb")
    nc.sync.dma_start(identb[:], identb_d[:])
    ea_sb = wp.tile([128, B], dt, tag="ea")
    nc.sync.dma_start(ea_sb[:], ea_d[:])
    masks_sb = wp.tile([B, T], F32, tag="masks")
    nc.sync.dma_start(masks_sb[:], masks[:])

    # ---- encoder: xT [300, BT] = W_enc.T @ inputs ( + b_enc ) ----
    # inputs_T streamed in n-slices; lhsT = W_enc k-chunk [kn, m-chunk]
    xT_sb = xp.tile([128, 3 * BTP], dt, tag="xT")          # m-chunks [128|128|44], t-major cols
    MCH = [(0, 128), (128, 128), (256, 44)]
    KCH = [(0, 128), (128, 128), (256, 104)]
    n_enc = min(512, BT)
    for n0 in range(0, BT, n_enc):
        insl = gp.tile([128, 3 * n_enc], dt, tag="inslice", bufs=2)
        for c, (r0, rn) in enumerate(KCH):
            nc.sync.dma_start(insl[0:rn, c * n_enc:(c + 1) * n_enc],
                              inT[r0:r0 + rn, n0:n0 + n_enc])
        for m, (m0, mn) in enumerate(MCH):
            ps = pp.tile([128, n_enc], F32, tag="enc_ps", bufs=1)
            for k, (k0, kn) in enumerate(KCH):
                nc.tensor.matmul(
                    ps[0:mn, :],
                    we_sb[0:kn, k * EMB + m0:k * EMB + m0 + mn],
                    insl[0:kn, k * n_enc:(k + 1) * n_enc],
                    start=(k == 0), stop=(k == 2))
            nc.scalar.activation(xT_sb[0:mn, m * BTP + n0:m * BTP + n0 + n_enc],
                                 ps[0:mn, :], AF.Identity,
                                 bias=benc_sb[0:mn, m:m + 1])
    # benc_sb chunk m holds b_enc[m0:m0+mn] at partitions [0:mn], col m.

    # ---- recurrence state tiles (persistent) ----
    h_sb = sp.tile([B, HID], dt, tag="h")                  # batch layout h
    hT_sb = sp.tile([128, 4 * B + MW], dt, tag="hT")       # k-layout + zero pad tail
    stA = sp.tile([128, HID], dt, tag="stackA")            # s1,s3,s5,s7 at bands 0,32,64,96
    stB = sp.tile([128, HID], dt, tag="stackB")            # s2,s4,s6,s8
    sT = [sp.tile([128, 4 * B + MW], WSDT, tag=f"sT{i}", name=f"sT{i}") for i in range(6)]  # s0..s5 k-layout + pad
    s0_sb = sp.tile([B, HID], dt, tag="s0")
    nc.sync.dma_start(h_sb[:], zeros_d[0:B, :])
    nc.sync.dma_start(hT_sb[:], zeros_d[:, 0:4 * B + MW])
    nc.sync.dma_start(stA[:], zeros_d[:])
    nc.sync.dma_start(stB[:], zeros_d[:])
    for _sti in range(6):
        nc.gpsimd.dma_start(sT[_sti][:, 4 * B:4 * B + MW], zeros_d[:, 0:MW])
    for _xc in range(3):
        nc.sync.dma_start(xT_sb[:, _xc * BTP + BT:(_xc + 1) * BTP], zeros_d[:, 0:B])

    out_stage = T  # DMA out every step directly

    def js_matmul(psum, lhs_chunks, w_tile, w_cols, n_total):
        """psum [32, n_total] at base 0. lhs_chunks: [kn, 32] APs (batch + pad);
        w_cols: base col of weight row-chunk k in w_tile."""
        for g in range(n_total // n_chunk):
            for k, lap in enumerate(lhs_chunks):
                kn = lap.shape[0]
                nc.tensor.matmul(
                    psum[0:32, g * n_chunk:(g + 1) * n_chunk],
                    lap, w_tile[0:kn, w_cols[k] + g * n_chunk:w_cols[k] + (g + 1) * n_chunk],
                    start=(k == 0), stop=(k == len(lhs_chunks) - 1))

    def gate(psum, act_name, inp_ap, off, si, t):
        """Gating for one connection. All SBUF gating tiles live at partition
        band [off:off+B] == the band of inp_ap, so SB+SB TensorTensor inputs
        share base partitions (walrus NCC_IBIR297).
        Returns (m_tile, off) for the transpose path."""
        sig = gp.tile([128, HID], dt, tag="sig")
        act = gp.tile([128, HID], dt, tag="act")
        m = gp.tile([128, HID], WSDT, tag="m")
        sg = sig[off:off + B, :]
        ag = act[off:off + B, :]
        mg = m[off:off + B, :]
        nc.scalar.activation(sg, psum[0:B, 0:HID], AF.Sigmoid)
        fn = AF.Copy if act_name == "identity" else ACT_FN[act_name]
        nc.scalar.activation(ag, psum[0:B, HID:NJS], fn)
        d = gp.tile([128, HID], dt, tag="d")
        dg = d[off:off + B, :]
        nc.vector.tensor_sub(dg, ag, inp_ap)
        nc.vector.tensor_mul(mg, sg, dg)
        st, soff = stack_pos(si)
        dst = (stA if st == 0 else stB)
        nc.vector.tensor_add(dst[soff:soff + B, :], mg, inp_ap)
        return m, off

    def transpose_state(m_tile, moff, parent_T, dst_T):
        """dst_T [128, 4B] = parent_T + m.T (4 PE transposes into one psum tile)."""
        mt_ps = pp.tile([128, 4 * B], WSDT, tag="mT")
        for c in range(4):
            nc.tensor.transpose(mt_ps[:, c * B:(c + 1) * B],
                                m_tile[moff:moff + B, c * 128:(c + 1) * 128],
                                identb[moff:moff + B, moff:moff + B],
                                tile_position=(moff, 0))
        nc.vector.tensor_add(dst_T[:, 0:4 * B], parent_T[:, 0:4 * B], mt_ps[:])

    W0_COLS = [c * NJS for c in range(7)]

    for t in range(T):
        # ---- initial cell: js0 = [x_t, h] @ W0 ----
        lhs = []
        for c, (r0, rn) in enumerate(XCH):
            # xT chunk c, t-major: cols [t*B : t*B + 32] (reads into next slice / pad)
            lhs.append(xT_sb[0:rn, c * BTP + t * B:c * BTP + t * B + MW])
        for c in range(4):
            lhs.append(hT_sb[:, c * B:c * B + MW])
        js0 = pjs.tile([32, NJS], F32, tag="js")
        js_matmul(js0, lhs, w0_sb, W0_COLS, NJS)
        # W0 gating: s0 = h + sig(c) * (tanh(g) - h)
        sig = gp.tile([B, HID], dt, tag="sig")
        act = gp.tile([B, HID], dt, tag="act")
        m0 = gp.tile([B, HID], WSDT, tag="m")
        nc.scalar.activation(sig[:], js0[0:B, 0:HID], AF.Sigmoid)
        nc.scalar.activation(act[:], js0[0:B, HID:NJS], AF.Tanh)
        d = gp.tile([B, HID], dt, tag="d")
        nc.vector.tensor_sub(d[:], act[:], h_sb[:])
        nc.vector.tensor_mul(m0[:], sig[:], d[:])
        nc.vector.tensor_add(s0_sb[:], m0[:], h_sb[:])
        transpose_state(m0, 0, hT_sb, sT[0])

        def sap(si):
            if si == 0:
                return s0_sb[:], 0
            st, off = stack_pos(si)
            return (stA if st == 0 else stB)[off:off + B, :], off

        for level in LEVELS:
            ms = []
            for i in level:
                act_name, conn = CONNECTIONS[i]
                jsp = pjs.tile([32, NJS], F32, tag="js")
                cols = [(i * 4 + c) * NJS for c in range(4)]
                js_matmul(jsp, [sT[conn][:, c * B:c * B + MW] for c in range(4)],
                          ws_sb, cols, NJS)
                inp_ap, ioff = sap(conn)
                m, moff = gate(jsp, act_name, inp_ap, ioff, i + 1, t)
                ms.append((i, m, moff))
            for i, m, moff in ms:
                if NEEDS_T[i + 1]:
                    transpose_state(m, moff, sT[CONNECTIONS[i][1]], sT[i + 1])

        # ---- h = mean(s1..s8) = EA.T @ stA + EA.T @ stB ----
        hp = pp.tile([B, HID], F32, tag="h_ps", bufs=1)
        nc.tensor.matmul(hp[:], ea_sb[:], stA[:], start=True, stop=False)
        nc.tensor.matmul(hp[:], ea_sb[:], stB[:], start=False, stop=True)
        # masked output + h copy
        ot = op.tile([B, HID], F32, tag="ot")
        nc.scalar.activation(ot[:], hp[:], AF.Copy, scale=masks_sb[:, t:t + 1])
        nc.sync.dma_start(out_d[:, t, :], ot[:])
        nc.vector.tensor_copy(h_sb[:], hp[:])
        # hT = transpose(h)
        ht_ps = pp.tile([128, 4 * B], DT, tag="mT")
        for c in range(4):
            nc.tensor.transpose(ht_ps[:, c * B:(c + 1) * B],
                                h_sb[:, c * 128:(c + 1) * 128], ident[:])
        nc.vector.tensor_copy(hT_sb[:, 0:4 * B], ht_ps[:])

    return nc


def build_full(B=16, T=256, n_chunk=256, n_cores=8):
    nc = bacc.Bacc("TRN2", target_bir_lowering=False, debug=False,
                   num_devices=n_cores)
    with tile.TileContext(nc) as tc:
        with ExitStack() as ctx:
            nc._build_ctx = ctx
            nc._build_tc = tc
            build(nc, B=B, T=T, n_chunk=n_chunk)
    nc.compile()
    return nc


def make_host_inputs(inputs, masks, W_enc, b_enc, W0, Ws, B_core, T):
    """Per-core in_maps from full inputs. inputs [B,T,360] fp32."""
    Bfull = inputs.shape[0]
    n_cores = Bfull // B_core
    npdt = mybir.dt.np(DT)
    npws = mybir.dt.np(WSDT)
    eye = np.eye(B_core, dtype=npdt)
    ea = np.zeros((128, B_core), dtype=npdt)
    for k in range(4):
        for b in range(B_core):
            ea[32 * k + b, b] = 0.125
    maps = []
    for c in range(n_cores):
        sl = slice(c * B_core, (c + 1) * B_core)
        inp = inputs[sl]                                  # [B, T, 360]
        inT = inp.transpose(1, 0, 2).reshape(T * B_core, IN_DIM).T.copy()  # [360, T*B], col = t*B+b
        maps.append({
            "inputs_T": np.ascontiguousarray(inT).astype(npdt),
            "masks": np.ascontiguousarray(masks[sl]).astype(np.float32),
            "W_enc": W_enc.astype(npdt), "b_enc": b_enc.astype(np.float32),
            "W0": W0.astype(npdt), "Ws": Ws.astype(npws),
            "ident": eye, "ident_bf": np.eye(128, dtype=npws), "EA": ea,
            "zeros": np.zeros((128, HID), dtype=npdt),
        })
    return maps


# ---------------- entry point ----------------
_CACHE = {}


def _get_nc():
    if "nc" not in _CACHE:
        _CACHE["nc"] = build_full(B=16, T=256, n_chunk=256, n_cores=8)
    return _CACHE["nc"]


def _run(maps, trace=False, **kw):
    from concourse.bass_utils import run_bass_kernel_spmd
    nc = _get_nc()
    return run_bass_kernel_spmd(nc, maps, list(range(8)), trace=trace, **kw)


def kernel(**inputs):
    inputs = {k: np.asarray(v) for k, v in inputs.items()}
    maps = make_host_inputs(
        inputs["inputs"].astype(np.float32),
        inputs["masks"].astype(np.float32),
        inputs["W_enc"].astype(np.float32),
        inputs["b_enc"].astype(np.float32),
        inputs["W0"].astype(np.float32),
        inputs["Ws"].astype(np.float32),
        B_core=16, T=256)
    res = _run(maps)
    out = np.concatenate([np.asarray(res.results[i]["out"]) for i in range(8)], axis=0)
    return out.astype(np.float32)

